# revision 1
# baseline (speedup 1.0000x reference)
"""Causal self-attention (B=4, S=2048, D=1024, H=16) on 8 NeuronCores.

Sharding: core c handles batch b = c//2 and head-group g = c%2 (8 heads).
Each core computes qkv for its head group, causal attention for its 8 heads,
and a partial projection (its 512 rows of W_proj). Host sums the two partial
outputs per batch and adds b_proj.

Device layout notes:
 - x is passed transposed (xT [D, S]) and bf16; qT/kT are computed in
   [qkv_col, token] layout so the scores matmul needs no transposes:
   scoresT[k_tok, q_tok] = kT_tile.T @ qT  (lhsT = kT, contraction = head dim).
 - softmax runs on scoresT: exp on ScalarE (scale=1/8 folded in); the causal
   mask is applied in PSUM by adding a 0/-1e9 triangular tile to the diagonal
   128-wide band via an identity matmul (PE), and the masked prefix of each
   k-tile row is simply never computed or accumulated.
 - denominators come from a ones-column appended to v (v_aug [k,65]); the
   ctx matmul then yields [ctx(64 rows); sums(1 row)] per q block.
 - normalization: reciprocal of the sums row, partition-broadcast on GpSimd,
   one fused multiply+cast on VectorE.
"""

import numpy as np
import ml_dtypes

import concourse.bacc as bacc
import concourse.tile as tile
from concourse import mybir
from concourse.bass_utils import run_bass_kernel_spmd

BF16 = mybir.dt.bfloat16
F32 = mybir.dt.float32
EXP = mybir.ActivationFunctionType.Exp

B = 4
S = 2048  # tokens per batch
D = 1024
HG = 8    # heads per core
HD = 64
GC = HG * HD  # 512 qkv columns per core per q/k/v
N_CORES = 8
SCALE = 0.125  # 1/sqrt(64)


def _body(nc, xT, wq, wk, wv, wp, bqkv, tri, ident, outT, tc, layout="fill", use_bias=True):
    _const_cm = tc.tile_pool(name="const", bufs=1)
    const = _const_cm.__enter__()
    qT_sb = const.tile([128, 4, S], BF16)
    kT_sb = const.tile([128, 4, S], BF16)
    ctxT_sb = const.tile([128, 4, S], BF16)
    vaug_sb = const.tile([128, 16, HG, 65], BF16)
    wp_sb = const.tile([128, 4, D], BF16)
    tri_sb = const.tile([128, 128], BF16)
    ident_sb = const.tile([128, 128], BF16)
    b_sb = const.tile([1, 3 * GC], BF16)
    ones1 = const.tile([1, 512], BF16)

    nc.vector.memset(ones1[:], 1.0)
    nc.vector.memset(vaug_sb[:, :, :, 64:65], 1.0)
    nc.sync.dma_start(out=tri_sb[:], in_=tri.ap())
    nc.sync.dma_start(out=ident_sb[:], in_=ident.ap())
    nc.sync.dma_start(out=b_sb[:], in_=bqkv.ap())
    for ct in range(4):
        nc.sync.dma_start(out=wp_sb[:, ct, :], in_=wp.ap()[128 * ct:128 * (ct + 1), :])

    xT_sb = const.tile([128, 8, S], BF16)
    wq_sb = const.tile([128, 8, GC], BF16)
    wk_sb = const.tile([128, 8, GC], BF16)
    wv_sb = const.tile([128, 8, GC], BF16)
    # xT + wq stream first so the first qk matmuls can start ASAP
    for t in range(8):
        nc.sync.dma_start(out=xT_sb[:, t, :], in_=xT.ap()[128 * t:128 * (t + 1), :])
        nc.sync.dma_start(out=wq_sb[:, t, :], in_=wq.ap()[128 * t:128 * (t + 1), :])
    for t in range(8):
        nc.sync.dma_start(out=wk_sb[:, t, :], in_=wk.ap()[128 * t:128 * (t + 1), :])
    for t in range(8):
        nc.sync.dma_start(out=wv_sb[:, t, :], in_=wv.ap()[128 * t:128 * (t + 1), :])

    # One shared PSUM pool scheme across all phases so emission can pipeline:
    #   scp "sc": [128,1024] slots x2 (4 banks) - qkv psums, scores, proj
    #   cxp "cx": [65,512] slots x4 (4 banks)   - ctx accumulators
    _scp_cm = tc.tile_pool(name="scp", bufs=2, space="PSUM")
    scp = _scp_cm.__enter__()
    _cxp_cm = tc.tile_pool(name="cxp", bufs=4, space="PSUM")
    cxp = _cxp_cm.__enter__()
    _prp_cm = tc.tile_pool(name="prp", bufs=6)
    prp = _prp_cm.__enter__()
    _nrm_cm = tc.tile_pool(name="nrm", bufs=4)
    nrm = _nrm_cm.__enter__()

    def qk_group(c, qk, tb):
        w_sb, dst, boff = ((wq_sb, qT_sb, 0), (wk_sb, kT_sb, GC))[qk]
        ps = scp.tile([128, 512], F32, tag="sc", name=f"qk_{c}_{boff}_{tb}")
        for t in range(8):
            nc.tensor.matmul(
                ps[:],
                lhsT=w_sb[:, t, 128 * c:128 * (c + 1)],
                rhs=xT_sb[:, t, 512 * tb:512 * (tb + 1)],
                start=(t == 0), stop=(not use_bias and t == 7))
        if use_bias:
            nc.tensor.matmul(
                ps[:],
                lhsT=b_sb[0:1, boff + 128 * c: boff + 128 * (c + 1)],
                rhs=ones1[0:1, :],
                start=False, stop=True)
        nc.vector.tensor_copy(dst[:, c, 512 * tb:512 * (tb + 1)], ps[:])

    def v_tile(j):
        # v in natural [token, v_col] layout, + bias, scattered into v_aug
        psv = scp.tile([128, 512], F32, tag="sc", name=f"pv_{j}")
        for t in range(8):
            nc.tensor.matmul(
                psv[:],
                lhsT=xT_sb[:, t, 128 * j:128 * (j + 1)],
                rhs=wv_sb[:, t, :],
                start=(t == 0), stop=(not use_bias and t == 7))
        if use_bias:
            nc.tensor.matmul(
                psv[:],
                lhsT=ones1[0:1, 0:128],
                rhs=b_sb[0:1, 2 * GC:3 * GC],
                start=False, stop=True)
        nc.vector.tensor_copy(
            vaug_sb[:, j, :, 0:64],
            psv[:].rearrange("p (h c) -> p h c", h=HG))

    def normalize(h, qb, ctx_ps):
        o = 64 * (h % 2)
        c = h // 2
        rec = nrm.tile([1, 512], F32, tag="rec", name=f"rec_{h}_{qb}")
        nc.vector.reciprocal(rec[:], ctx_ps[qb][64:65, :])
        bc = nrm.tile([64, 512], F32, tag="bc", name=f"bc_{h}_{qb}")
        nc.gpsimd.partition_broadcast(bc[:], rec[:])
        if o == 0:
            nc.vector.tensor_mul(
                ctxT_sb[0:64, c, 512 * qb:512 * (qb + 1)],
                ctx_ps[qb][0:64, :], bc[:])
        else:
            stg = nrm.tile([64, 512], BF16, tag="stg", name=f"stg_{h}_{qb}")
            nc.vector.tensor_mul(stg[:], ctx_ps[qb][0:64, :], bc[:])
            nc.sync.dma_start(
                out=ctxT_sb[64:128, c, 512 * qb:512 * (qb + 1)], in_=stg[:])

    _ob3_cm = tc.tile_pool(name="ob3", bufs=4)
    ob3 = _ob3_cm.__enter__()

    def proj_group(m, tb):
        ps = scp.tile([128, 512], F32, tag="sc", name=f"p3_{m}_{tb}")
        for ct in range(4):
            nc.tensor.matmul(
                ps[:],
                lhsT=wp_sb[:, ct, 128 * m:128 * (m + 1)],
                rhs=ctxT_sb[:, ct, 512 * tb:512 * (tb + 1)],
                start=(ct == 0), stop=(ct == 3))
        ob = ob3.tile([128, 512], F32, tag="o3", name=f"ob_{m}_{tb}")
        # ACT is idle by the time the projection runs; keep DVE free
        nc.scalar.copy(ob[:], ps[:])
        nc.sync.dma_start(
            out=outT.ap()[128 * m:128 * (m + 1), 512 * tb:512 * (tb + 1)],
            in_=ob[:])

    def h7_filler(j):
        # tb-block tb of the projection becomes legal once head 7's q-block
        # tb is normalized at j = 4*tb + 3; emit 2 (m, tb) groups per j
        if j >= 4:
            idx = j - 4
            tb, pair = idx // 4, idx % 4
            proj_group(2 * pair, tb)
            proj_group(2 * pair + 1, tb)

    def head_block(h, filler=None):
        o = 64 * (h % 2)
        c = h // 2
        ctx_ps = [cxp.tile([65, 512], F32, tag="cx", name=f"cx_{h}_{qb}")
                  for qb in range(4)]
        for j in range(16):
            if filler is not None:
                filler(j)
            qbm, r = divmod(j, 4)
            width = S - 512 * qbm
            rel0 = 128 * r
            pT = prp.tile([128, S], BF16, tag="probs", name=f"pT_{h}_{j}")
            # scores chunks of <=1024 free, one exp per chunk; the causal mask
            # is applied in PSUM by adding tri_neg (0 / -1e9) to the diagonal
            # 128-wide band via an identity matmul, keeping the whole
            # scores->exp chain on PE->ACT only
            for ch0 in range(0, width, 1024):
                ch1 = min(ch0 + 1024, width)
                lo = max(ch0, rel0)
                if lo >= ch1:
                    continue
                ps = scp.tile([128, 1024], F32, tag="sc", name=f"sc_{h}_{j}_{ch0}")
                for qb in range(qbm + ch0 // 512, qbm + ch1 // 512):
                    rq0 = (qb - qbm) * 512
                    mlo = max(rq0, rel0)
                    diag = mlo == rel0 and ch0 == 0
                    nc.tensor.matmul(
                        ps[:, mlo - ch0: rq0 + 512 - ch0],
                        lhsT=kT_sb[o:o + 64, c, 128 * j:128 * (j + 1)],
                        rhs=qT_sb[o:o + 64, c,
                                  512 * qbm + mlo: 512 * qbm + rq0 + 512],
                        start=True, stop=not diag, skip_group_check=True)
                    if diag:
                        nc.tensor.matmul(
                            ps[:, rel0 - ch0: rel0 - ch0 + 128],
                            lhsT=ident_sb[:],
                            rhs=tri_sb[:],
                            start=False, stop=True, skip_group_check=True)
                nc.scalar.activation(
                    pT[:, lo:ch1], ps[:, lo - ch0:ch1 - ch0], EXP, scale=SCALE)
            # ctx accumulation (with sums in row 64); the diagonal block's
            # masked prefix [0, rel0) is never computed nor accumulated
            for qb in range(qbm, 4):
                lo = rel0 if qb == qbm else 0
                nc.tensor.matmul(
                    ctx_ps[qb][:, lo:512],
                    lhsT=vaug_sb[:, j, h, :],
                    rhs=pT[:, (qb - qbm) * 512 + lo: (qb - qbm + 1) * 512],
                    start=(j == 0), stop=(j == 4 * qb + 3))
            if r == 3:
                # qb = (j-3)//4 just received its last accumulation
                normalize(h, (j - 3) // 4, ctx_ps)

    def spread(groups):
        stride = max(1, 16 // max(1, len(groups)))
        def f(j):
            i = j // stride
            if j % stride == 0 and i < len(groups):
                groups[i]()
        return f

    qkg = [[(lambda c=c, qk=qk, tb=tb: qk_group(c, qk, tb))
            for qk in range(2) for tb in range(4)] for c in range(4)]
    if layout == "fill":
        # qk(0) upfront; v interleaved into h0 two iterations ahead of use;
        # qk(1..3) spread into h1..h5
        for g in qkg[0]:
            g()
        v_tile(0)
        v_tile(1)
        head_block(0, filler=lambda j: v_tile(j + 2) if j < 14 else None)
        head_block(1, filler=spread(qkg[1]))
        head_block(2, filler=spread(qkg[2][:4]))
        head_block(3, filler=spread(qkg[2][4:]))
        head_block(4, filler=spread(qkg[3][:4]))
        head_block(5, filler=spread(qkg[3][4:]))
        head_block(6)
        head_block(7, filler=h7_filler)
    elif layout == "seq":
        # all qkv upfront, then pure attention heads
        for c in range(4):
            for g in qkg[c]:
                g()
        for j in range(16):
            v_tile(j)
        for h in range(HG - 1):
            head_block(h)
        head_block(7, filler=h7_filler)
    elif layout == "block":
        # qkv blocks between head pairs
        for g in qkg[0]:
            g()
        for j in range(16):
            v_tile(j)
        for c in range(4):
            if c:
                for g in qkg[c]:
                    g()
            head_block(2 * c)
            head_block(2 * c + 1, filler=h7_filler if c == 3 else None)
    else:
        raise ValueError(layout)


    for pair in range(4):
        proj_group(2 * pair, 3)
        proj_group(2 * pair + 1, 3)

    _ob3_cm.__exit__(None, None, None)
    _nrm_cm.__exit__(None, None, None)
    _prp_cm.__exit__(None, None, None)
    _cxp_cm.__exit__(None, None, None)
    _scp_cm.__exit__(None, None, None)
    _const_cm.__exit__(None, None, None)


_CACHED = {}


def _build(reps=1, layout="fill", use_bias=True):
    key = (reps, layout, use_bias)
    if key in _CACHED:
        return _CACHED[key]
    nc = bacc.Bacc()
    xT = nc.dram_tensor("xT", [D, S], BF16, kind="ExternalInput")
    wq = nc.dram_tensor("wq", [D, GC], BF16, kind="ExternalInput")
    wk = nc.dram_tensor("wk", [D, GC], BF16, kind="ExternalInput")
    wv = nc.dram_tensor("wv", [D, GC], BF16, kind="ExternalInput")
    wp = nc.dram_tensor("wp", [GC, D], BF16, kind="ExternalInput")
    bqkv = nc.dram_tensor("bqkv", [1, 3 * GC], BF16, kind="ExternalInput")
    tri = nc.dram_tensor("tri", [128, 128], BF16, kind="ExternalInput")
    ident = nc.dram_tensor("ident", [128, 128], BF16, kind="ExternalInput")
    outT = nc.dram_tensor("outT", [D, S], F32, kind="ExternalOutput")
    with tile.TileContext(nc) as tc:
        for _ in range(reps):
            _body(nc, xT, wq, wk, wv, wp, bqkv, tri, ident, outT, tc, layout=layout, use_bias=use_bias)
    nc.compile()
    _CACHED[key] = nc
    return nc


def make_in_maps(x, W_attn, b_attn, W_proj):
    bf = ml_dtypes.bfloat16
    tri_np = np.where(np.arange(128)[None, :] >= np.arange(128)[:, None],
                      np.float32(0.0), np.float32(-1e9)).astype(bf)
    ident_np = np.eye(128, dtype=np.float32).astype(bf)
    in_maps = []
    for core in range(N_CORES):
        b, g = divmod(core, 2)
        cols = slice(GC * g, GC * (g + 1))
        in_maps.append({
            "xT": np.ascontiguousarray(x[b].T).astype(bf),
            "wq": np.ascontiguousarray(W_attn[:, cols]).astype(bf),
            "wk": np.ascontiguousarray(W_attn[:, D:][:, cols]).astype(bf),
            "wv": np.ascontiguousarray(W_attn[:, 2 * D:][:, cols]).astype(bf),
            "wp": np.ascontiguousarray(W_proj[cols, :]).astype(bf),
            "bqkv": np.concatenate(
                [b_attn[cols], b_attn[D:][cols], b_attn[2 * D:][cols]]
            ).reshape(1, 3 * GC).astype(bf),
            "tri": tri_np,
            "ident": ident_np,
        })
    return in_maps


def kernel(x, W_attn, b_attn, W_proj, b_proj, _run_kwargs=None):
    x = np.asarray(x)
    W_attn = np.asarray(W_attn)
    b_attn = np.asarray(b_attn)
    W_proj = np.asarray(W_proj)
    b_proj = np.asarray(b_proj)

    use_bias = bool(np.any(b_attn))
    nc = _build(use_bias=use_bias)
    in_maps = make_in_maps(x, W_attn, b_attn, W_proj)

    res = run_bass_kernel_spmd(
        nc, in_maps, core_ids=list(range(N_CORES)), **(_run_kwargs or {}))

    out = np.empty((B, S, D), np.float32)
    for b in range(B):
        acc = res.results[2 * b]["outT"] + res.results[2 * b + 1]["outT"]
        out[b] = acc.T + b_proj[None, :].astype(np.float32)
    if _run_kwargs:
        kernel.last_results = res
    return out



# revision 32
# speedup vs baseline: 534.7336x; 534.7336x over previous
"""Causal self-attention (B=4, S=2048, D=1024, H=16) on 8 NeuronCores.

Sharding: core c handles batch b = c//2 and head-group g = c%2 (8 heads).
Each core computes qkv for its head group, causal attention for its 8 heads,
and a partial projection (its 512 rows of W_proj). Host sums the two partial
outputs per batch and adds b_proj.

Device layout notes:
 - x is passed transposed (xT [D, S]) and bf16; qT/kT are computed in
   [qkv_col, token] layout so the scores matmul needs no transposes:
   scoresT[k_tok, q_tok] = kT_tile.T @ qT  (lhsT = kT, contraction = head dim).
 - softmax runs on scoresT: exp on ScalarE (scale=1/8 folded in); the causal
   mask is applied in PSUM by adding a 0/-1e9 triangular tile to the diagonal
   128-wide band via an identity matmul (PE), and the masked prefix of each
   k-tile row is simply never computed or accumulated.
 - denominators come from a ones-column appended to v (v_aug [k,65]); the
   ctx matmul then yields [ctx(64 rows); sums(1 row)] per q block.
 - normalization: reciprocal of the sums row, partition-broadcast on GpSimd,
   one fused multiply+cast on VectorE.
"""

import numpy as np
import ml_dtypes

import concourse.bacc as bacc
import concourse.tile as tile
from concourse import mybir
from concourse.bass_utils import run_bass_kernel_spmd

BF16 = mybir.dt.bfloat16
F32 = mybir.dt.float32
I16 = mybir.dt.int16
F8 = mybir.dt.float8e4
DR = mybir.MatmulPerfMode.DoubleRow
EXP = mybir.ActivationFunctionType.Exp

B = 4
S = 2048  # tokens per batch
D = 1024
HG = 8    # heads per core
HD = 64
GC = HG * HD  # 512 qkv columns per core per q/k/v
N_CORES = 8
SCALE = 0.125  # 1/sqrt(64)

# Free scale slots (exp scale, normalize multiply, proj copy) can absorb a
# host-side weight pre-scale; unused (1.0) in the bf16 configuration.
WS = 1.0

# DVE fast-exp (Schraudolph): I = trunc(A_EXP*s + B_EXP) as int16, bitcast to
# bf16 gives ~exp(s*SCALE) with ~1.8% rms error. Masked scores (-1e9 added in
# PSUM) drive I to int16 saturation/wrap, whose bf16 bitcast is +-0.0 — i.e.
# the mask falls out for free on this path too.
A_EXP = 128.0 * 1.4426950408889634 * SCALE / (WS * WS)
B_EXP = 16256.0 - 6.9
EXP_SCALE = SCALE / (WS * WS)

# debug knobs (affect _body emission; not part of the build cache key, so only
# flip these in fresh processes). FAST_RECIP (reciprocal_approx_fast on the
# psum sums row) and TAIL_OPT (tail proj via the cx psum pool) both verified
# broken on hardware despite passing CoreSim — keep them off.
DVE_EXP = True
CTX_DELAY = True
TAIL_OPT = False
FAST_RECIP = False


def _body(nc, xT, wq, wk, wv, wp, bqkv, tri, ident, outT, tc, layout="fill", use_bias=True):
    _const_cm = tc.tile_pool(name="const", bufs=1)
    const = _const_cm.__enter__()
    qT_sb = const.tile([128, 4, S], BF16)
    kT_sb = const.tile([128, 4, S], BF16)
    ctxT_sb = const.tile([128, 4, S], BF16)
    vaug_sb = const.tile([128, 16, HG, 65], BF16)
    wp_sb = const.tile([128, 4, D], BF16)
    tri_sb = const.tile([128, 128], BF16)
    ident_sb = const.tile([128, 128], BF16)
    b_sb = const.tile([1, 3 * GC], BF16)
    ones1 = const.tile([1, 512], BF16)

    nc.vector.memset(ones1[:], 1.0)
    nc.vector.memset(vaug_sb[:, :, :, 64:65], 1.0)
    nc.sync.dma_start(out=tri_sb[:], in_=tri.ap())
    nc.sync.dma_start(out=ident_sb[:], in_=ident.ap())
    nc.sync.dma_start(out=b_sb[:], in_=bqkv.ap())
    for ct in range(4):
        nc.sync.dma_start(out=wp_sb[:, ct, :], in_=wp.ap()[128 * ct:128 * (ct + 1), :])

    xT_sb = const.tile([128, 8, S], BF16)
    wq_sb = const.tile([128, 8, GC], BF16)
    wk_sb = const.tile([128, 8, GC], BF16)
    wv_sb = const.tile([128, 8, GC], BF16)
    # xT + wq stream first so the first qk matmuls can start ASAP
    for t in range(8):
        nc.sync.dma_start(out=xT_sb[:, t, :], in_=xT.ap()[128 * t:128 * (t + 1), :])
        nc.sync.dma_start(out=wq_sb[:, t, :], in_=wq.ap()[128 * t:128 * (t + 1), :])
    for t in range(8):
        nc.sync.dma_start(out=wk_sb[:, t, :], in_=wk.ap()[128 * t:128 * (t + 1), :])
    for t in range(8):
        nc.sync.dma_start(out=wv_sb[:, t, :], in_=wv.ap()[128 * t:128 * (t + 1), :])

    # One shared PSUM pool scheme across all phases so emission can pipeline:
    #   scp "sc": [128,1024] slots x2 (4 banks) - qkv psums, scores, proj
    #   cxp "cx": [65,512] slots x4 (4 banks)   - ctx accumulators
    _scp_cm = tc.tile_pool(name="scp", bufs=2, space="PSUM")
    scp = _scp_cm.__enter__()
    _cxp_cm = tc.tile_pool(name="cxp", bufs=4, space="PSUM")
    cxp = _cxp_cm.__enter__()
    _prp_cm = tc.tile_pool(name="prp", bufs=6)
    prp = _prp_cm.__enter__()
    _nrm_cm = tc.tile_pool(name="nrm", bufs=4)
    nrm = _nrm_cm.__enter__()

    def qk_group(c, qk, tb):
        w_sb, dst, boff = ((wq_sb, qT_sb, 0), (wk_sb, kT_sb, GC))[qk]
        ps = scp.tile([128, 512], F32, tag="sc", name=f"qk_{c}_{boff}_{tb}")
        for t in range(8):
            nc.tensor.matmul(
                ps[:],
                lhsT=w_sb[:, t, 128 * c:128 * (c + 1)],
                rhs=xT_sb[:, t, 512 * tb:512 * (tb + 1)],
                start=(t == 0), stop=(not use_bias and t == 7))
        if use_bias:
            nc.tensor.matmul(
                ps[:],
                lhsT=b_sb[0:1, boff + 128 * c: boff + 128 * (c + 1)],
                rhs=ones1[0:1, :],
                start=False, stop=True)
        nc.vector.tensor_copy(dst[:, c, 512 * tb:512 * (tb + 1)], ps[:])

    def v_tile(j):
        # v in natural [token, v_col] layout, + bias, scattered into v_aug
        psv = scp.tile([128, 512], F32, tag="sc", name=f"pv_{j}")
        for t in range(8):
            nc.tensor.matmul(
                psv[:],
                lhsT=xT_sb[:, t, 128 * j:128 * (j + 1)],
                rhs=wv_sb[:, t, :],
                start=(t == 0), stop=(not use_bias and t == 7))
        if use_bias:
            nc.tensor.matmul(
                psv[:],
                lhsT=ones1[0:1, 0:128],
                rhs=b_sb[0:1, 2 * GC:3 * GC],
                start=False, stop=True)
        nc.vector.tensor_copy(
            vaug_sb[:, j, :, 0:64],
            psv[:].rearrange("p (h c) -> p h c", h=HG))

    def normalize(h, qb, ctx_ps):
        o = 64 * (h % 2)
        c = h // 2
        rec = nrm.tile([1, 512], F32, tag="rec", name=f"rec_{h}_{qb}")
        if FAST_RECIP:
            nc.vector.reciprocal_approx_fast(rec[:], ctx_ps[qb][64:65, :])
        else:
            nc.vector.reciprocal(rec[:], ctx_ps[qb][64:65, :])
        bc = nrm.tile([64, 512], F32, tag="bc", name=f"bc_{h}_{qb}")
        nc.gpsimd.partition_broadcast(bc[:], rec[:])
        if o == 0:
            nc.vector.tensor_mul(
                ctxT_sb[0:64, c, 512 * qb:512 * (qb + 1)],
                ctx_ps[qb][0:64, :], bc[:])
        else:
            stg = nrm.tile([64, 512], BF16, tag="stg", name=f"stg_{h}_{qb}")
            nc.vector.tensor_mul(stg[:], ctx_ps[qb][0:64, :], bc[:])
            nc.sync.dma_start(
                out=ctxT_sb[64:128, c, 512 * qb:512 * (qb + 1)], in_=stg[:])

    _ob3_cm = tc.tile_pool(name="ob3", bufs=4)
    ob3 = _ob3_cm.__enter__()

    def proj_group(m, tb, tail=False):
        # tail groups alternate onto the (by then idle) ctx psum pool and
        # copy on DVE, so neither psum-slot reuse nor ACT queueing gates PE
        pool, tag = (cxp, "cx") if (tail and m % 2 and TAIL_OPT) else (scp, "sc")
        ps = pool.tile([128, 512], F32, tag=tag, name=f"p3_{m}_{tb}")
        for ct in range(4):
            nc.tensor.matmul(
                ps[:],
                lhsT=wp_sb[:, ct, 128 * m:128 * (m + 1)],
                rhs=ctxT_sb[:, ct, 512 * tb:512 * (tb + 1)],
                start=(ct == 0), stop=(ct == 3))
        ob = ob3.tile([128, 512], F32, tag="o3", name=f"ob_{m}_{tb}")
        if tail and m % 2 and TAIL_OPT:
            nc.vector.tensor_copy(ob[:], ps[:])
        else:
            nc.scalar.copy(ob[:], ps[:])
        nc.sync.dma_start(
            out=outT.ap()[128 * m:128 * (m + 1), 512 * tb:512 * (tb + 1)],
            in_=ob[:])

    def h7_filler(j):
        # tb-block tb of the projection becomes legal once the last head's
        # q-block tb is normalized, which (with ctx delayed one j) is emitted
        # inside iteration j = 4*tb + 4; emit 2 (m, tb) groups per j from
        # j = 4*tb + 5 on, leaving the leftovers for the tail
        if j >= 5:
            idx = j - 5
            tb, pair = idx // 4, idx % 4
            proj_group(2 * pair, tb)
            proj_group(2 * pair + 1, tb)

    def emit_exp(dst, src, dve):
        if dve:
            # Schraudolph fast exp: one DVE pass, int16 result bitcast to bf16
            nc.vector.tensor_scalar(
                out=dst.bitcast(I16), in0=src,
                scalar1=A_EXP, scalar2=B_EXP,
                op0=mybir.AluOpType.mult, op1=mybir.AluOpType.add)
        else:
            nc.scalar.activation(dst, src, EXP, scale=EXP_SCALE)

    def head_block(h, filler=None, dve_sel=None):
        o = 64 * (h % 2)
        c = h // 2
        ctx_ps = [cxp.tile([65, 512], F32, tag="cx", name=f"cx_{h}_{qb}")
                  for qb in range(4)]
        pT_tiles = {}

        def ctx_mms(j):
            # ctx accumulation (with sums in row 64); the diagonal block's
            # masked prefix [0, rel0) is never computed nor accumulated.
            # Emitted one j late so these matmuls overlap exp(j+1).
            qbm, r = divmod(j, 4)
            rel0 = 128 * r
            pT = pT_tiles.pop(j)
            for qb in range(qbm, 4):
                lo = rel0 if qb == qbm else 0
                nc.tensor.matmul(
                    ctx_ps[qb][:, lo:512],
                    lhsT=vaug_sb[:, j, h, :],
                    rhs=pT[:, (qb - qbm) * 512 + lo: (qb - qbm + 1) * 512],
                    start=(j == 0), stop=(j == 4 * qb + 3))
            if r == 3:
                # qb = (j-3)//4 just received its last accumulation
                normalize(h, (j - 3) // 4, ctx_ps)

        for j in range(16):
            if filler is not None:
                filler(j)
            qbm, r = divmod(j, 4)
            width = S - 512 * qbm
            rel0 = 128 * r
            pT = prp.tile([128, S], BF16, tag="probs", name=f"pT_{h}_{j}")
            pT_tiles[j] = pT
            # scores chunks of <=1024 free, one exp per chunk; the causal mask
            # is applied in PSUM by adding tri_neg (0 / -1e9) to the diagonal
            # 128-wide band via an identity matmul; exp of each chunk runs on
            # ACT or DVE per dve_sel to balance the two engines
            for ch0 in range(0, width, 1024):
                ch1 = min(ch0 + 1024, width)
                lo = max(ch0, rel0)
                if lo >= ch1:
                    continue
                ps = scp.tile([128, 1024], F32, tag="sc", name=f"sc_{h}_{j}_{ch0}")
                for qb in range(qbm + ch0 // 512, qbm + ch1 // 512):
                    rq0 = (qb - qbm) * 512
                    mlo = max(rq0, rel0)
                    diag = mlo == rel0 and ch0 == 0
                    nc.tensor.matmul(
                        ps[:, mlo - ch0: rq0 + 512 - ch0],
                        lhsT=kT_sb[o:o + 64, c, 128 * j:128 * (j + 1)],
                        rhs=qT_sb[o:o + 64, c,
                                  512 * qbm + mlo: 512 * qbm + rq0 + 512],
                        start=True, stop=not diag, skip_group_check=True)
                    if diag:
                        nc.tensor.matmul(
                            ps[:, rel0 - ch0: rel0 - ch0 + 128],
                            lhsT=ident_sb[:],
                            rhs=tri_sb[:],
                            start=False, stop=True, skip_group_check=True)
                # default routing: off-diagonal chunks go to DVE, except near
                # head boundaries (j<=0 or j>=14) where DVE must stay free for
                # the normalize chain
                dve = (dve_sel(h, j, ch0) if dve_sel is not None
                       else (ch0 > 0 and 1 <= j <= 13)) and DVE_EXP
                emit_exp(pT[:, lo:ch1], ps[:, lo - ch0:ch1 - ch0], dve)
            if CTX_DELAY:
                if j > 0:
                    ctx_mms(j - 1)
            else:
                ctx_mms(j)
        if CTX_DELAY:
            ctx_mms(15)

    def spread(groups):
        stride = max(1, 16 // max(1, len(groups)))
        def f(j):
            i = j // stride
            if j % stride == 0 and i < len(groups):
                groups[i]()
        return f

    qkg = [[(lambda c=c, qk=qk, tb=tb: qk_group(c, qk, tb))
            for qk in range(2) for tb in range(4)] for c in range(4)]
    if layout == "fill":
        # qk(0) upfront; v interleaved into h0 two iterations ahead of use;
        # qk(1..3) spread into h1..h5. Head 7 (odd — its normalize ends in a
        # staging DMA) runs before head 6 so the tail-critical last head is
        # even, whose normalize chain is shorter.
        for g in qkg[0]:
            g()
        v_tile(0)
        v_tile(1)
        head_block(0, filler=lambda j: v_tile(j + 2) if j < 14 else None)
        head_block(1, filler=spread(qkg[1]))
        head_block(2, filler=spread(qkg[2][:4]))
        head_block(3, filler=spread(qkg[2][4:]))
        head_block(4, filler=spread(qkg[3][:4]))
        head_block(5, filler=spread(qkg[3][4:]))
        head_block(7)
        head_block(6, filler=h7_filler)
    elif layout == "seq":
        # all qkv upfront, then pure attention heads
        for c in range(4):
            for g in qkg[c]:
                g()
        for j in range(16):
            v_tile(j)
        for h in range(HG - 1):
            head_block(h)
        head_block(7, filler=h7_filler)
    elif layout == "block":
        # qkv blocks between head pairs
        for g in qkg[0]:
            g()
        for j in range(16):
            v_tile(j)
        for c in range(4):
            if c:
                for g in qkg[c]:
                    g()
            head_block(2 * c)
            head_block(2 * c + 1, filler=h7_filler if c == 3 else None)
    else:
        raise ValueError(layout)


    # leftovers h7_filler couldn't place; the tb=2 groups are independent
    # of the last normalize and overlap the tail chain latency
    proj_group(6, 2, tail=True)
    proj_group(7, 2, tail=True)
    for pair in range(4):
        proj_group(2 * pair, 3, tail=True)
        proj_group(2 * pair + 1, 3, tail=True)

    _ob3_cm.__exit__(None, None, None)
    _nrm_cm.__exit__(None, None, None)
    _prp_cm.__exit__(None, None, None)
    _cxp_cm.__exit__(None, None, None)
    _scp_cm.__exit__(None, None, None)
    _const_cm.__exit__(None, None, None)


_CACHED = {}


def _build(reps=1, layout="fill", use_bias=True):
    key = (reps, layout, use_bias)
    if key in _CACHED:
        return _CACHED[key]
    nc = bacc.Bacc()
    xT = nc.dram_tensor("xT", [D, S], BF16, kind="ExternalInput")
    wq = nc.dram_tensor("wq", [D, GC], BF16, kind="ExternalInput")
    wk = nc.dram_tensor("wk", [D, GC], BF16, kind="ExternalInput")
    wv = nc.dram_tensor("wv", [D, GC], BF16, kind="ExternalInput")
    wp = nc.dram_tensor("wp", [GC, D], BF16, kind="ExternalInput")
    bqkv = nc.dram_tensor("bqkv", [1, 3 * GC], BF16, kind="ExternalInput")
    tri = nc.dram_tensor("tri", [128, 128], BF16, kind="ExternalInput")
    ident = nc.dram_tensor("ident", [128, 128], BF16, kind="ExternalInput")
    outT = nc.dram_tensor("outT", [D, S], F32, kind="ExternalOutput")
    with tile.TileContext(nc) as tc:
        for _ in range(reps):
            _body(nc, xT, wq, wk, wv, wp, bqkv, tri, ident, outT, tc, layout=layout, use_bias=use_bias)
    nc.compile()
    _CACHED[key] = nc
    return nc


def make_in_maps(x, W_attn, b_attn, W_proj):
    bf = ml_dtypes.bfloat16
    f8 = mybir.dt.np(F8)
    tri_np = np.where(np.arange(128)[None, :] >= np.arange(128)[:, None],
                      np.float32(0.0), np.float32(-1e9)).astype(bf)
    ident_np = np.eye(128, dtype=np.float32).astype(bf)
    in_maps = []
    for core in range(N_CORES):
        b, g = divmod(core, 2)
        cols = slice(GC * g, GC * (g + 1))
        in_maps.append({
            "xT": np.ascontiguousarray(x[b].T).astype(bf),
            "wq": np.ascontiguousarray(W_attn[:, cols]).astype(bf),
            "wk": np.ascontiguousarray(W_attn[:, D:][:, cols]).astype(bf),
            "wv": np.ascontiguousarray(W_attn[:, 2 * D:][:, cols]).astype(bf),
            "wp": np.ascontiguousarray(W_proj[cols, :]).astype(bf),
            "bqkv": np.concatenate(
                [b_attn[cols], b_attn[D:][cols], b_attn[2 * D:][cols]]
            ).reshape(1, 3 * GC).astype(bf),
            "tri": tri_np,
            "ident": ident_np,
        })
    return in_maps


def kernel(x, W_attn, b_attn, W_proj, b_proj, _run_kwargs=None):
    x = np.asarray(x)
    W_attn = np.asarray(W_attn)
    b_attn = np.asarray(b_attn)
    W_proj = np.asarray(W_proj)
    b_proj = np.asarray(b_proj)

    use_bias = bool(np.any(b_attn))
    nc = _build(use_bias=use_bias)
    in_maps = make_in_maps(x, W_attn, b_attn, W_proj)

    res = run_bass_kernel_spmd(
        nc, in_maps, core_ids=list(range(N_CORES)), **(_run_kwargs or {}))

    out = np.empty((B, S, D), np.float32)
    for b in range(B):
        acc = res.results[2 * b]["outT"] + res.results[2 * b + 1]["outT"]
        out[b] = acc.T + b_proj[None, :].astype(np.float32)
    if _run_kwargs:
        kernel.last_results = res
    return out



# revision 42
# speedup vs baseline: 598.3542x; 1.1190x over previous
"""Causal self-attention (B=4, S=2048, D=1024, H=16) on 8 NeuronCores.

Sharding: core c handles batch b = c//2 and head-group g = c%2 (8 heads).
Each core computes qkv for its head group, causal attention for its 8 heads,
and a partial projection (its 512 rows of W_proj). Host sums the two partial
outputs per batch and adds b_proj.

Device layout notes:
 - x is passed transposed (xT [D, S]) and bf16; qT/kT are computed in
   [qkv_col, token] layout so the scores matmul needs no transposes:
   scoresT[k_tok, q_tok] = kT_tile.T @ qT  (lhsT = kT, contraction = head dim).
 - softmax runs on scoresT: the causal mask is applied in PSUM by adding a
   0/-1e9 triangular tile to the diagonal 128-wide band via an identity
   matmul (PE); the masked prefix of each k-tile row is never computed.
 - exp is split across TWO engines to unthrottle the scores->exp->ctx chain:
   diagonal chunks run on ScalarE (table exp, scale folded in); off-diagonal
   chunks run on VectorE as a one-instruction Schraudolph fast exp
   (I = rint(a*s + b) as int16, bitcast to bf16; ~1.8% rms, which softmax
   renormalization mostly cancels - end-to-end rel err 6.5e-3 vs 5.4e-3 for
   exact exp). Masked entries saturate the int16 and bitcast to -0.0.
 - ctx matmuls for k-tile j are emitted one j late, so they execute while
   exp(j+1) is still running on ACT/DVE - removes the per-j PE stall and the
   head-boundary stall on the normalize chain (PSUM-neutral reorder).
 - denominators come from a ones-column appended to v (v_aug [k,65]); the
   ctx matmul then yields [ctx(64 rows); sums(1 row)] per q block.
 - normalization: reciprocal of the sums row, partition-broadcast on GpSimd,
   one fused multiply+cast on VectorE.

Measured (8 axon trn2 cores): 318 us vs 387 us baseline; CoreSim 291 us with
PE 80% busy (233 us of matmul). Known-bad variants (pass CoreSim, fail HW):
reciprocal_approx_fast on the [1,512] psum sums row at base partition 64, and
tail proj psums allocated from the [65,512]-sized cx pool. fp8 DoubleRow for
qkv/proj is fast but numerically dead here: quantization noise in a zero-mean
random contraction does NOT average out (rel err stays at per-element ~4%),
giving 7e-2 end-to-end vs the 2e-2 budget.
"""

import numpy as np
import ml_dtypes

import concourse.bacc as bacc
import concourse.tile as tile
from concourse import mybir
from concourse.bass_utils import run_bass_kernel_spmd

BF16 = mybir.dt.bfloat16
F32 = mybir.dt.float32
I16 = mybir.dt.int16
F8 = mybir.dt.float8e4
DR = mybir.MatmulPerfMode.DoubleRow
EXP = mybir.ActivationFunctionType.Exp

B = 4
S = 2048  # tokens per batch
D = 1024
HG = 8    # heads per core
HD = 64
GC = HG * HD  # 512 qkv columns per core per q/k/v
N_CORES = 8
SCALE = 0.125  # 1/sqrt(64)

# Free scale slots (exp scale, normalize multiply, proj copy) can absorb a
# host-side weight pre-scale; unused (1.0) in the bf16 configuration.
WS = 1.0

# DVE fast-exp (Schraudolph): I = trunc(A_EXP*s + B_EXP) as int16, bitcast to
# bf16 gives ~exp(s*SCALE) with ~1.8% rms error. Masked scores (-1e9 added in
# PSUM) drive I to int16 saturation/wrap, whose bf16 bitcast is +-0.0 — i.e.
# the mask falls out for free on this path too.
A_EXP = 128.0 * 1.4426950408889634 * SCALE / (WS * WS)
B_EXP = 16256.0 - 6.9
EXP_SCALE = SCALE / (WS * WS)

# debug knobs (affect _body emission; not part of the build cache key, so only
# flip these in fresh processes). FAST_RECIP (reciprocal_approx_fast on the
# psum sums row) and TAIL_OPT (tail proj via the cx psum pool) both verified
# broken on hardware despite passing CoreSim — keep them off.
DVE_EXP = True
CTX_DELAY = True
TAIL_OPT = False
FAST_RECIP = False


def _body(nc, xT, wq, wk, wv, wp, bqkv, tri, ident, outT, tc, layout="fill", use_bias=True):
    _const_cm = tc.tile_pool(name="const", bufs=1)
    const = _const_cm.__enter__()
    qT_sb = const.tile([128, 4, S], BF16)
    kT_sb = const.tile([128, 4, S], BF16)
    ctxT_sb = const.tile([128, 4, S], BF16)
    vaug_sb = const.tile([128, 16, HG, 65], BF16)
    wp_sb = const.tile([128, 4, D], BF16)
    tri_sb = const.tile([128, 128], BF16)
    ident_sb = const.tile([128, 128], BF16)
    b_sb = const.tile([1, 3 * GC], BF16)
    ones1 = const.tile([1, 512], BF16)

    nc.vector.memset(ones1[:], 1.0)
    nc.vector.memset(vaug_sb[:, :, :, 64:65], 1.0)
    # pin the exp table set while the input DMAs stream: the first ACT
    # instruction pays the ~1.3us ACT_TABLE_LOAD, and an early Copy would
    # otherwise load a different set and force a reload at the first real exp
    dummy = const.tile([1, 2], F32)
    nc.vector.memset(dummy[:], 0.0)
    nc.scalar.activation(dummy[0:1, 0:1], dummy[0:1, 1:2], EXP, scale=1.0)
    nc.sync.dma_start(out=tri_sb[:], in_=tri.ap())
    nc.sync.dma_start(out=ident_sb[:], in_=ident.ap())
    nc.sync.dma_start(out=b_sb[:], in_=bqkv.ap())
    for ct in range(4):
        nc.sync.dma_start(out=wp_sb[:, ct, :], in_=wp.ap()[128 * ct:128 * (ct + 1), :])

    xT_sb = const.tile([128, 8, S], BF16)
    wq_sb = const.tile([128, 8, GC], BF16)
    wk_sb = const.tile([128, 8, GC], BF16)
    wv_sb = const.tile([128, 8, GC], BF16)
    # xT + wq stream first so the first qk matmuls can start ASAP
    for t in range(8):
        nc.sync.dma_start(out=xT_sb[:, t, :], in_=xT.ap()[128 * t:128 * (t + 1), :])
        nc.sync.dma_start(out=wq_sb[:, t, :], in_=wq.ap()[128 * t:128 * (t + 1), :])
    for t in range(8):
        nc.sync.dma_start(out=wk_sb[:, t, :], in_=wk.ap()[128 * t:128 * (t + 1), :])
    for t in range(8):
        nc.sync.dma_start(out=wv_sb[:, t, :], in_=wv.ap()[128 * t:128 * (t + 1), :])

    # One shared PSUM pool scheme across all phases so emission can pipeline:
    #   scp "sc": [128,1024] slots x2 (4 banks) - qkv psums, scores, proj
    #   cxp "cx": [65,512] slots x4 (4 banks)   - ctx accumulators
    _scp_cm = tc.tile_pool(name="scp", bufs=2, space="PSUM")
    scp = _scp_cm.__enter__()
    _cxp_cm = tc.tile_pool(name="cxp", bufs=4, space="PSUM")
    cxp = _cxp_cm.__enter__()
    _prp_cm = tc.tile_pool(name="prp", bufs=6)
    prp = _prp_cm.__enter__()
    _nrm_cm = tc.tile_pool(name="nrm", bufs=4)
    nrm = _nrm_cm.__enter__()

    def qk_group(c, qk, tb):
        w_sb, dst, boff = ((wq_sb, qT_sb, 0), (wk_sb, kT_sb, GC))[qk]
        ps = scp.tile([128, 512], F32, tag="sc", name=f"qk_{c}_{boff}_{tb}")
        for t in range(8):
            nc.tensor.matmul(
                ps[:],
                lhsT=w_sb[:, t, 128 * c:128 * (c + 1)],
                rhs=xT_sb[:, t, 512 * tb:512 * (tb + 1)],
                start=(t == 0), stop=(not use_bias and t == 7))
        if use_bias:
            nc.tensor.matmul(
                ps[:],
                lhsT=b_sb[0:1, boff + 128 * c: boff + 128 * (c + 1)],
                rhs=ones1[0:1, :],
                start=False, stop=True)
        nc.vector.tensor_copy(dst[:, c, 512 * tb:512 * (tb + 1)], ps[:])

    def qk_quad(items):
        # t-outer across 4 qk groups: each xT/w chunk t is consumed by all
        # four groups as soon as it lands, so the startup matmuls pipeline
        # with the DMA stream instead of blocking on the last chunk per group.
        # The two [128,1024] slots host two groups each (one bank per group).
        pss = [scp.tile([128, 1024], F32, tag="sc", name=f"qq{i}_{items[i][2]}")
               for i in range(2)]
        aps = [pss[i // 2][:, 512 * (i % 2):512 * (i % 2 + 1)] for i in range(4)]
        for t in range(8):
            for gi, (c, qk, tb) in enumerate(items):
                w_sb = (wq_sb, wk_sb)[qk]
                nc.tensor.matmul(
                    aps[gi],
                    lhsT=w_sb[:, t, 128 * c:128 * (c + 1)],
                    rhs=xT_sb[:, t, 512 * tb:512 * (tb + 1)],
                    start=(t == 0), stop=(not use_bias and t == 7),
                    skip_group_check=True)
        for gi, (c, qk, tb) in enumerate(items):
            boff = (0, GC)[qk]
            if use_bias:
                nc.tensor.matmul(
                    aps[gi],
                    lhsT=b_sb[0:1, boff + 128 * c: boff + 128 * (c + 1)],
                    rhs=ones1[0:1, :],
                    start=False, stop=True, skip_group_check=True)
            dst = (qT_sb, kT_sb)[qk]
            # split copies across ACT/DVE — both are idle during startup and
            # a single engine doing all 4 serializes the first scores matmuls
            if gi % 2:
                nc.scalar.copy(dst[:, c, 512 * tb:512 * (tb + 1)], aps[gi])
            else:
                nc.vector.tensor_copy(dst[:, c, 512 * tb:512 * (tb + 1)], aps[gi])

    def v_tile(j):
        # v in natural [token, v_col] layout, + bias, scattered into v_aug
        psv = scp.tile([128, 512], F32, tag="sc", name=f"pv_{j}")
        for t in range(8):
            nc.tensor.matmul(
                psv[:],
                lhsT=xT_sb[:, t, 128 * j:128 * (j + 1)],
                rhs=wv_sb[:, t, :],
                start=(t == 0), stop=(not use_bias and t == 7))
        if use_bias:
            nc.tensor.matmul(
                psv[:],
                lhsT=ones1[0:1, 0:128],
                rhs=b_sb[0:1, 2 * GC:3 * GC],
                start=False, stop=True)
        nc.vector.tensor_copy(
            vaug_sb[:, j, :, 0:64],
            psv[:].rearrange("p (h c) -> p h c", h=HG))

    def normalize(h, qb, ctx_ps):
        o = 64 * (h % 2)
        c = h // 2
        rec = nrm.tile([1, 512], F32, tag="rec", name=f"rec_{h}_{qb}")
        if FAST_RECIP:
            nc.vector.reciprocal_approx_fast(rec[:], ctx_ps[qb][64:65, :])
        else:
            nc.vector.reciprocal(rec[:], ctx_ps[qb][64:65, :])
        bc = nrm.tile([64, 512], F32, tag="bc", name=f"bc_{h}_{qb}")
        nc.gpsimd.partition_broadcast(bc[:], rec[:])
        if o == 0:
            nc.vector.tensor_mul(
                ctxT_sb[0:64, c, 512 * qb:512 * (qb + 1)],
                ctx_ps[qb][0:64, :], bc[:])
        else:
            stg = nrm.tile([64, 512], BF16, tag="stg", name=f"stg_{h}_{qb}")
            nc.vector.tensor_mul(stg[:], ctx_ps[qb][0:64, :], bc[:])
            nc.sync.dma_start(
                out=ctxT_sb[64:128, c, 512 * qb:512 * (qb + 1)], in_=stg[:])

    _ob3_cm = tc.tile_pool(name="ob3", bufs=4)
    ob3 = _ob3_cm.__enter__()

    def proj_group(m, tb, tail=False):
        # tail groups alternate onto the (by then idle) ctx psum pool and
        # copy on DVE, so neither psum-slot reuse nor ACT queueing gates PE
        pool, tag = (cxp, "cx") if (tail and m % 2 and TAIL_OPT) else (scp, "sc")
        ps = pool.tile([128, 512], F32, tag=tag, name=f"p3_{m}_{tb}")
        for ct in range(4):
            nc.tensor.matmul(
                ps[:],
                lhsT=wp_sb[:, ct, 128 * m:128 * (m + 1)],
                rhs=ctxT_sb[:, ct, 512 * tb:512 * (tb + 1)],
                start=(ct == 0), stop=(ct == 3))
        ob = ob3.tile([128, 512], F32, tag="o3", name=f"ob_{m}_{tb}")
        if tail and m % 2:
            # alternate copy engines in the tail so two queued ACT copies
            # never gate the psum slot ring
            nc.vector.tensor_copy(ob[:], ps[:])
        else:
            nc.scalar.copy(ob[:], ps[:])
        nc.sync.dma_start(
            out=outT.ap()[128 * m:128 * (m + 1), 512 * tb:512 * (tb + 1)],
            in_=ob[:])

    def h7_filler(j):
        # tb-block tb of the projection becomes legal once the last head's
        # q-block tb is normalized, which (with ctx delayed two j) is emitted
        # inside iteration j = 4*tb + 5; emit 2 (m, tb) groups per j from
        # j = 4*tb + 6 on, leaving the leftovers for the tail
        if j >= 6:
            idx = j - 6
            tb, pair = idx // 4, idx % 4
            proj_group(2 * pair, tb)
            proj_group(2 * pair + 1, tb)

    def emit_exp(dst, src, dve):
        if dve:
            # Schraudolph fast exp: one DVE pass, int16 result bitcast to bf16
            nc.vector.tensor_scalar(
                out=dst.bitcast(I16), in0=src,
                scalar1=A_EXP, scalar2=B_EXP,
                op0=mybir.AluOpType.mult, op1=mybir.AluOpType.add)
        else:
            nc.scalar.activation(dst, src, EXP, scale=EXP_SCALE)

    def head_block(h, filler=None, dve_sel=None):
        o = 64 * (h % 2)
        c = h // 2
        ctx_ps = [cxp.tile([65, 512], F32, tag="cx", name=f"cx_{h}_{qb}")
                  for qb in range(4)]
        pT_tiles = {}

        def ctx_mms(j):
            # ctx accumulation (with sums in row 64); the diagonal block's
            # masked prefix [0, rel0) is never computed nor accumulated.
            # Emitted one j late so these matmuls overlap exp(j+1).
            qbm, r = divmod(j, 4)
            rel0 = 128 * r
            pT = pT_tiles.pop(j)
            for qb in range(qbm, 4):
                lo = rel0 if qb == qbm else 0
                nc.tensor.matmul(
                    ctx_ps[qb][:, lo:512],
                    lhsT=vaug_sb[:, j, h, :],
                    rhs=pT[:, (qb - qbm) * 512 + lo: (qb - qbm + 1) * 512],
                    start=(j == 0), stop=(j == 4 * qb + 3))
            if r == 3:
                # qb = (j-3)//4 just received its last accumulation
                normalize(h, (j - 3) // 4, ctx_ps)

        for j in range(16):
            if filler is not None:
                filler(j)
            qbm, r = divmod(j, 4)
            width = S - 512 * qbm
            rel0 = 128 * r
            pT = prp.tile([128, S], BF16, tag="probs", name=f"pT_{h}_{j}")
            pT_tiles[j] = pT
            # scores chunks of <=1024 free, one exp per chunk; the causal mask
            # is applied in PSUM by adding tri_neg (0 / -1e9) to the diagonal
            # 128-wide band via an identity matmul; exp of each chunk runs on
            # ACT or DVE per dve_sel to balance the two engines
            for ch0 in range(0, width, 1024):
                ch1 = min(ch0 + 1024, width)
                lo = max(ch0, rel0)
                if lo >= ch1:
                    continue
                ps = scp.tile([128, 1024], F32, tag="sc", name=f"sc_{h}_{j}_{ch0}")
                for qb in range(qbm + ch0 // 512, qbm + ch1 // 512):
                    rq0 = (qb - qbm) * 512
                    mlo = max(rq0, rel0)
                    diag = mlo == rel0 and ch0 == 0
                    nc.tensor.matmul(
                        ps[:, mlo - ch0: rq0 + 512 - ch0],
                        lhsT=kT_sb[o:o + 64, c, 128 * j:128 * (j + 1)],
                        rhs=qT_sb[o:o + 64, c,
                                  512 * qbm + mlo: 512 * qbm + rq0 + 512],
                        start=True, stop=not diag, skip_group_check=True)
                    if diag:
                        nc.tensor.matmul(
                            ps[:, rel0 - ch0: rel0 - ch0 + 128],
                            lhsT=ident_sb[:],
                            rhs=tri_sb[:],
                            start=False, stop=True, skip_group_check=True)
                # default routing: off-diagonal chunks go to DVE, except near
                # head boundaries (j<=1 or j>=14) where DVE must stay free for
                # the previous head's normalize chain
                dve = (dve_sel(h, j, ch0) if dve_sel is not None
                       else (ch0 > 0 and 2 <= j <= 13)) and DVE_EXP
                emit_exp(pT[:, lo:ch1], ps[:, lo - ch0:ch1 - ch0], dve)
            if CTX_DELAY:
                # two-j-deep pipeline: ctx(j-2) runs while exp(j-1)/exp(j)
                # compute, covering even the wide early-j exps and the
                # scores-psum WAR on the slot ring
                if j > 1:
                    ctx_mms(j - 2)
            else:
                ctx_mms(j)
        if CTX_DELAY:
            ctx_mms(14)
            ctx_mms(15)

    def spread(groups):
        stride = max(1, 16 // max(1, len(groups)))
        def f(j):
            i = j // stride
            if j % stride == 0 and i < len(groups):
                groups[i]()
        return f

    qkg = [[(lambda c=c, qk=qk, tb=tb: qk_group(c, qk, tb))
            for qk in range(2) for tb in range(4)] for c in range(4)]
    if layout == "fill":
        # qk(0) upfront; v interleaved into h0 two iterations ahead of use;
        # qk(1..3) spread into h1..h5. Head 7 (odd — its normalize ends in a
        # staging DMA) runs before head 6 so the tail-critical last head is
        # even, whose normalize chain is shorter.
        qk_quad([(0, 0, tb) for tb in range(4)])
        qk_quad([(0, 1, tb) for tb in range(4)])
        v_tile(0)
        v_tile(1)
        head_block(0, filler=lambda j: v_tile(j + 2) if j < 14 else None)
        head_block(1, filler=spread(qkg[1]))
        head_block(2, filler=spread(qkg[2][:4]))
        head_block(3, filler=spread(qkg[2][4:]))
        head_block(4, filler=spread(qkg[3][:4]))
        head_block(5, filler=spread(qkg[3][4:]))
        head_block(7)
        head_block(6, filler=h7_filler)
    elif layout == "seq":
        # all qkv upfront, then pure attention heads
        for c in range(4):
            for g in qkg[c]:
                g()
        for j in range(16):
            v_tile(j)
        for h in range(HG - 1):
            head_block(h)
        head_block(7, filler=h7_filler)
    elif layout == "block":
        # qkv blocks between head pairs
        for g in qkg[0]:
            g()
        for j in range(16):
            v_tile(j)
        for c in range(4):
            if c:
                for g in qkg[c]:
                    g()
            head_block(2 * c)
            head_block(2 * c + 1, filler=h7_filler if c == 3 else None)
    else:
        raise ValueError(layout)


    # leftovers h7_filler couldn't place; the tb=2 groups are independent
    # of the last normalize and overlap the tail chain latency
    proj_group(4, 2, tail=True)
    proj_group(5, 2, tail=True)
    proj_group(6, 2, tail=True)
    proj_group(7, 2, tail=True)
    for pair in range(4):
        proj_group(2 * pair, 3, tail=True)
        proj_group(2 * pair + 1, 3, tail=True)

    _ob3_cm.__exit__(None, None, None)
    _nrm_cm.__exit__(None, None, None)
    _prp_cm.__exit__(None, None, None)
    _cxp_cm.__exit__(None, None, None)
    _scp_cm.__exit__(None, None, None)
    _const_cm.__exit__(None, None, None)


_CACHED = {}


def _build(reps=1, layout="fill", use_bias=True):
    key = (reps, layout, use_bias)
    if key in _CACHED:
        return _CACHED[key]
    nc = bacc.Bacc()
    xT = nc.dram_tensor("xT", [D, S], BF16, kind="ExternalInput")
    wq = nc.dram_tensor("wq", [D, GC], BF16, kind="ExternalInput")
    wk = nc.dram_tensor("wk", [D, GC], BF16, kind="ExternalInput")
    wv = nc.dram_tensor("wv", [D, GC], BF16, kind="ExternalInput")
    wp = nc.dram_tensor("wp", [GC, D], BF16, kind="ExternalInput")
    bqkv = nc.dram_tensor("bqkv", [1, 3 * GC], BF16, kind="ExternalInput")
    tri = nc.dram_tensor("tri", [128, 128], BF16, kind="ExternalInput")
    ident = nc.dram_tensor("ident", [128, 128], BF16, kind="ExternalInput")
    outT = nc.dram_tensor("outT", [D, S], F32, kind="ExternalOutput")
    with tile.TileContext(nc) as tc:
        for _ in range(reps):
            _body(nc, xT, wq, wk, wv, wp, bqkv, tri, ident, outT, tc, layout=layout, use_bias=use_bias)
    nc.compile()
    _CACHED[key] = nc
    return nc


def make_in_maps(x, W_attn, b_attn, W_proj):
    bf = ml_dtypes.bfloat16
    f8 = mybir.dt.np(F8)
    tri_np = np.where(np.arange(128)[None, :] >= np.arange(128)[:, None],
                      np.float32(0.0), np.float32(-1e9)).astype(bf)
    ident_np = np.eye(128, dtype=np.float32).astype(bf)
    in_maps = []
    for core in range(N_CORES):
        b, g = divmod(core, 2)
        cols = slice(GC * g, GC * (g + 1))
        in_maps.append({
            "xT": np.ascontiguousarray(x[b].T).astype(bf),
            "wq": np.ascontiguousarray(W_attn[:, cols]).astype(bf),
            "wk": np.ascontiguousarray(W_attn[:, D:][:, cols]).astype(bf),
            "wv": np.ascontiguousarray(W_attn[:, 2 * D:][:, cols]).astype(bf),
            "wp": np.ascontiguousarray(W_proj[cols, :]).astype(bf),
            "bqkv": np.concatenate(
                [b_attn[cols], b_attn[D:][cols], b_attn[2 * D:][cols]]
            ).reshape(1, 3 * GC).astype(bf),
            "tri": tri_np,
            "ident": ident_np,
        })
    return in_maps


def kernel(x, W_attn, b_attn, W_proj, b_proj, _run_kwargs=None):
    x = np.asarray(x)
    W_attn = np.asarray(W_attn)
    b_attn = np.asarray(b_attn)
    W_proj = np.asarray(W_proj)
    b_proj = np.asarray(b_proj)

    use_bias = bool(np.any(b_attn))
    nc = _build(use_bias=use_bias)
    in_maps = make_in_maps(x, W_attn, b_attn, W_proj)

    res = run_bass_kernel_spmd(
        nc, in_maps, core_ids=list(range(N_CORES)), **(_run_kwargs or {}))

    out = np.empty((B, S, D), np.float32)
    for b in range(B):
        acc = res.results[2 * b]["outT"] + res.results[2 * b + 1]["outT"]
        out[b] = acc.T + b_proj[None, :].astype(np.float32)
    if _run_kwargs:
        kernel.last_results = res
    return out

